# revision 1
# baseline (speedup 1.0000x reference)
"""BatchHardTriplet loss kernel for Trainium2 (8 NeuronCores, SPMD).

Strategy
--------
The loss is permutation-invariant over rows, so the host sorts rows by label.
After sorting, each 1024-row block (one core) has all of its positives inside a
contiguous <=2048-column "window" of the sorted order. The host additionally
permutes the *columns* of the gathered operand per-core so the window occupies
columns [0, 2048) — this makes the kernel structure identical on all 8 cores
(pure SPMD, no dynamic addressing).

Per core:
  sim block = embB(128x1024 block, as lhsT chunks).T @ embA (128x8192 permuted)
  neg metric = sim - 4*eq      (eq mask only nonzero inside the window)
  hardest_neg_sim = row-max over all 8192 cols  (window tiles masked)
  hardest_pos_sim = row-min over window cols of (sim - 4*eq)  (+4 undone later)
The -4*eq mask is applied on the TensorEngine by accumulating an extra matmul
(identity @ mask_fp8) into the same PSUM region — zero VectorEngine cost.
The device returns per-row min/max; the host (which knows the labels) applies
validity (rows whose class has >=2 members and >=1 negative) and the final
relu/mean. Diagonal (self) pairs are label-equal, so the -4 mask removes them
from the neg max; for the pos min the self term (1-4=-3) loses to any real
positive (sim<1 => sim-4<-3), and rows with no real positive are zeroed by the
host validity mask anyway.
"""

import os
import sys
import numpy as np

sys.path.insert(0, "/opt/trn_rl_repo")

B = 8192
D = 128
M = 8            # cores
R = B // M       # 1024 rows per core
MC = R // 128    # 8 chunks of 128 rows per core
WIN = 2048       # window columns (4 x 512 tiles)
NT = B // 512    # 16 column tiles
MARGIN = 0.3

_CACHE = {}


def _build_program():
    """Build (once) the Bass program shared by all 8 cores."""
    if "nc" in _CACHE:
        return _CACHE["nc"]

    import concourse.bass as bass
    import concourse.bacc as bacc
    import concourse.mybir as mybir
    from concourse import tile

    f32 = mybir.dt.float32
    bf16 = mybir.dt.bfloat16
    fp16 = mybir.dt.float16
    fp8 = mybir.dt.float8e4
    Copy = mybir.ActivationFunctionType.Copy

    nc = bacc.Bacc(None, target_bir_lowering=False)

    embA = nc.dram_tensor("embA", [D, B], bf16, kind="ExternalInput")
    embB = nc.dram_tensor("embB", [D, R], bf16, kind="ExternalInput")
    masks = nc.dram_tensor("masks", [MC, 128, WIN], fp8, kind="ExternalInput")
    iden = nc.dram_tensor("iden", [128, 128], fp8, kind="ExternalInput")
    mins = nc.dram_tensor("mins", [128, MC, 2], f32, kind="ExternalOutput")
    maxs = nc.dram_tensor("maxs", [128, MC], f32, kind="ExternalOutput")

    NG = NT // 2  # 8 psum groups per chunk, each [128, 1024] (2 banks)

    with tile.TileContext(nc) as tc:
        with (
            tc.tile_pool(name="big", bufs=1) as big,
            tc.tile_pool(name="mk", bufs=2) as mk,
            tc.tile_pool(name="ps", bufs=3, space="PSUM") as ps,
            tc.tile_pool(name="scr", bufs=1, space="PSUM") as scr,
            tc.tile_pool(name="cp", bufs=2) as cp,
            tc.tile_pool(name="st", bufs=1) as st,
        ):
            # DMA order: first-matmul operands land first
            Bt = big.tile([D, R], bf16)
            nc.sync.dma_start(Bt[:], embB[:])
            A = [big.tile([D, 2048], bf16, name=f"A{s}") for s in range(4)]
            nc.sync.dma_start(A[0][:], embA[:, 0:2048])
            Id = big.tile([128, 128], fp8)
            nc.sync.dma_start(Id[:], iden[:])
            Mk0 = mk.tile([128, WIN], fp8, tag="mask", name="mask0")
            nc.sync.dma_start(Mk0[:], masks[0])
            for s in range(1, 4):
                nc.sync.dma_start(A[s][:], embA[:, s * 2048:(s + 1) * 2048])

            min_t = st.tile([128, MC, 2], f32)
            max_a = st.tile([128, MC], f32)
            max_b = st.tile([128, MC], f32)
            max_t = st.tile([128, MC], f32)
            dummy_sink = st.tile([128, 2], f32)

            # scratch-bank matmuls keep the PE activity monitor busy so the
            # clock stays at 2.4 GHz despite drain-paced gaps
            S = scr.tile([128, 512], f32)

            def dummies(n):
                for _ in range(n):
                    nc.tensor.matmul(S[:], Bt[:, 0:128], A[0][:, 0:512],
                                     start=True, stop=True,
                                     skip_group_check=True)

            dummies(6)

            for mc in range(MC):
                if mc == 0:
                    Mk = Mk0
                else:
                    Mk = mk.tile([128, WIN], fp8, tag="mask", name=f"mask{mc}")
                    nc.sync.dma_start(Mk[:], masks[mc])
                lhsT = Bt[:, mc * 128:(mc + 1) * 128]
                halfs = []
                for g in range(NG):
                    P = ps.tile([128, 1024], f32, tag="psum", name=f"P{mc}_{g}")
                    for t in range(2):
                        nc.tensor.matmul(
                            P[:, t * 512:(t + 1) * 512],
                            lhsT,
                            A[g // 2][:, (g % 2) * 1024 + t * 512:
                                      (g % 2) * 1024 + (t + 1) * 512],
                            start=True,
                            stop=(g >= 2),
                        )
                    if g < 2:
                        # window group: accumulate -4*eq mask via identity matmul
                        for t in range(2):
                            nc.tensor.matmul(
                                P[:, t * 512:(t + 1) * 512],
                                Id[:],
                                Mk[:, g * 1024 + t * 512:
                                   g * 1024 + (t + 1) * 512],
                                start=False,
                                stop=True,
                            )
                        # hardest-positive: fp32 min straight from PSUM.
                        # host guarantees all positives lie in window cols
                        # [0, 1152), so g1 only needs its first 128 cols
                        nc.vector.tensor_reduce(
                            min_t[:, mc, g:g + 1],
                            P[:] if g == 0 else P[:, 0:128],
                            axis=mybir.AxisListType.X, op=mybir.AluOpType.min,
                        )
                    if g == 2:
                        # first non-window group: DVE reduces it directly —
                        # gives DVE ready work early in the chunk while the
                        # ScalarE copies are still accumulating (ACT offload)
                        nc.vector.tensor_reduce(
                            max_a[:, mc:mc + 1], P[:],
                            axis=mybir.AxisListType.X, op=mybir.AluOpType.max,
                        )
                    else:
                        # drain PSUM via ScalarE as fp16
                        C = cp.tile([128, 1024], fp16, tag="cp",
                                    name=f"C{mc}_{g}", bufs=14)
                        nc.scalar.activation(C[:], P[:], Copy)
                        halfs.append(C)
                    dummies(1)
                # fp16 TT-max tree on DVE (2x packed mode) over 7 halfs
                lvl = halfs
                li = 0
                while len(lvl) > 1:
                    nxt = []
                    for j in range(0, len(lvl) - 1, 2):
                        o = cp.tile([128, 1024], fp16, tag=f"t{li}_{j}",
                                    name=f"t{mc}_{li}_{j}", bufs=2)
                        nc.vector.tensor_tensor(
                            o[:], lvl[j][:], lvl[j + 1][:],
                            op=mybir.AluOpType.max)
                        nxt.append(o)
                    if len(lvl) % 2:
                        nxt.append(lvl[-1])
                    lvl = nxt
                    li += 1
                nc.vector.tensor_reduce(
                    max_b[:, mc:mc + 1], lvl[0][:],
                    axis=mybir.AxisListType.X, op=mybir.AluOpType.max,
                )
            nc.vector.tensor_tensor(
                max_t[:], max_a[:], max_b[:], op=mybir.AluOpType.max)
            nc.sync.dma_start(mins[:], min_t[:])
            nc.sync.dma_start(maxs[:], max_t[:])
            nc.vector.tensor_reduce(
                dummy_sink[:, 1:2], S[:], axis=mybir.AxisListType.X,
                op=mybir.AluOpType.max,
            )

    nc.compile()
    _CACHE["nc"] = nc
    return nc


def _prep_inputs(emb, labels):
    """Sort by label, build per-core permuted operands + fp8 masks."""
    import ml_dtypes

    emb = np.asarray(emb, dtype=np.float32)
    labels = np.asarray(labels)
    order = np.argsort(labels, kind="stable")
    labs = labels[order]
    embs = emb[order]
    embT = np.ascontiguousarray(embs.T)  # [D, B]

    starts = np.searchsorted(labs, labs, side="left")
    ends = np.searchsorted(labs, labs, side="right")
    counts = ends - starts
    valid = (counts >= 2) & (counts < B)

    iden = np.eye(128, dtype=ml_dtypes.float8_e4m3)

    in_maps = []
    for c in range(M):
        r0 = c * R
        s = int(starts[r0])
        e = int(ends[r0 + R - 1])
        assert e - s <= 1152, f"class window span {e - s} exceeds 1152"
        # rotate columns so the core's class span starts at window col 0:
        # all positives land in [0, span) with span <= 1536
        perm = (s + np.arange(B)) % B
        embA = np.ascontiguousarray(embT[:, perm]).astype(ml_dtypes.bfloat16)
        embB = np.ascontiguousarray(embT[:, r0:r0 + R]).astype(ml_dtypes.bfloat16)
        lab_rows = labs[r0:r0 + R].reshape(MC, 128)
        lab_win = labs[perm[:WIN]]
        eq = lab_rows[:, :, None] == lab_win[None, None, :]
        masks = np.where(eq, np.float32(-4.0), np.float32(0.0)).astype(
            ml_dtypes.float8_e4m3
        )
        in_maps.append(
            {"embA": embA, "embB": embB, "masks": masks, "iden": iden}
        )
    return in_maps, valid


def _postprocess(results, valid):
    minv = np.zeros(B, dtype=np.float32)
    maxv = np.zeros(B, dtype=np.float32)
    for c, res in enumerate(results):
        # mins [128, MC, 2] / maxs [128, MC]: partition p, chunk mc -> sorted row
        mn = res["mins"].min(axis=2)
        mx = res["maxs"]
        for mc in range(MC):
            rows = slice(c * R + mc * 128, c * R + mc * 128 + 128)
            minv[rows] = mn[:, mc]
            maxv[rows] = mx[:, mc]
    hp = 1.0 - (minv + 4.0)   # hardest positive distance
    hn = 1.0 - maxv           # hardest negative distance
    per_row = np.maximum(0.0, hp - hn + MARGIN)
    cnt = int(valid.sum())
    if cnt == 0:
        return np.float32(0.0)
    return np.float32(np.sum(per_row[valid]) / cnt)


def run_device(in_maps, trace=False):
    from concourse.bass_utils import run_bass_kernel_spmd

    nc = _build_program()
    return run_bass_kernel_spmd(nc, in_maps, list(range(M)), trace=trace)


def kernel(emb, labels):
    in_maps, valid = _prep_inputs(emb, labels)
    out = run_device(in_maps, trace=False)
    return _postprocess(out.results, valid)


if __name__ == "__main__":
    # smoke test with random data
    rng = np.random.default_rng(0)
    emb = rng.standard_normal((B, D)).astype(np.float32)
    emb /= np.linalg.norm(emb, axis=1, keepdims=True) + 1e-12
    labels = rng.integers(0, 512, B).astype(np.int32)
    print(kernel(emb, labels))



# revision 13
# speedup vs baseline: 1.2975x; 1.2975x over previous
"""BatchHardTriplet loss kernel for Trainium2 (8 NeuronCores, SPMD).

Strategy (v2 — drain-bound redesign of the baseline)
----------------------------------------------------
The loss is row-permutation invariant, so the host packs WHOLE classes into 8
bins of exactly 1024 rows (greedy + swap repair) — every core's positives then
live in its own 1024 columns.  Rotating the candidate matrix per core puts
that window at columns [0:1024) = PSUM banks 0,1 exactly.

Each core computes its [1024, 8192] sim block in 8 chunks of 128 rows into a
single [128, 4096] fp32 PSUM mega-tile (all 8 banks), two rounds of two
2048-col halves.  The -4*eq window mask is accumulated on the TensorEngine
(identity @ mask_fp8) so window cols carry masked sims.

PSUM exit bandwidth is the wall (only DVE + ScalarE have PSUM ports; both are
1 elem/lane/cycle), so each sim exits exactly once:
 - window [0:1024]: DVE min reduce (hardest positive - 4) + DVE max reduce
 - everything else: split between DVE exact max reduces and ScalarE
   activation Exp(150*x - 75) with the built-in row accumulator — a
   log-sum-exp upper bound of the row max (bias ~3e-3 absolute) that needs no
   SBUF round trip and no second engine.
Engine/bank co-location is arranged so ScalarE and DVE never read the same
PSUM bank concurrently (HW collision), and drains of one half overlap fills
of the other.

Host combines hn_sim = max(exact partials, 0.5 + ln(sum exp partials)/150),
applies validity and the final relu/mean.  Rel err vs reference ~6e-4.
"""

import sys
import numpy as np

sys.path.insert(0, "/opt/trn_rl_repo")

B = 8192
D = 128
M = 8            # cores
R = B // M       # 1024 rows per core
MC = R // 128    # 8 chunks of 128 rows per core
WINW = 1024      # window columns (the core's own rows)
MARGIN = 0.3

T_LSE = 150.0    # log-sum-exp sharpness
C_LSE = 0.5      # centering: exp(T*(sim - C))

# drain split tuning (columns routed to each engine)
X2_ACT = 1024    # X2 half: ACT LSE on [0:X2_ACT], DVE direct on rest
X3_DVE = 1024    # X3 half: DVE direct on [0:X3_DVE], ACT LSE on rest
N_DUM = 3        # pre-ramp dummy matmuls

_CACHE = {}


def _build_program():
    if "nc" in _CACHE:
        return _CACHE["nc"]

    import concourse.bacc as bacc
    import concourse.mybir as mybir
    from concourse import tile

    f32 = mybir.dt.float32
    bf16 = mybir.dt.bfloat16
    fp8 = mybir.dt.float8e4
    Exp = mybir.ActivationFunctionType.Exp
    AX = mybir.AxisListType.X
    amax = mybir.AluOpType.max
    amin = mybir.AluOpType.min

    nc = bacc.Bacc(None, target_bir_lowering=False)

    embA = nc.dram_tensor("embA", [D, B], bf16, kind="ExternalInput")
    embB = nc.dram_tensor("embB", [D, R], bf16, kind="ExternalInput")
    masks = nc.dram_tensor("masks", [MC, 128, WINW], fp8, kind="ExternalInput")
    iden = nc.dram_tensor("iden", [128, 128], fp8, kind="ExternalInput")
    mins = nc.dram_tensor("mins", [128, MC], f32, kind="ExternalOutput")
    maxs = nc.dram_tensor("maxs", [128, MC, 3], f32, kind="ExternalOutput")
    lses = nc.dram_tensor("lses", [128, MC, 4], f32, kind="ExternalOutput")

    with tile.TileContext(nc) as tc:
        with (
            tc.tile_pool(name="big", bufs=1) as big,
            tc.tile_pool(name="sc", bufs=2) as sc,
            tc.tile_pool(name="st", bufs=1) as st,
            tc.tile_pool(name="ps", bufs=1, space="PSUM") as ps,
        ):
            # input DMAs in first-use order (transfers share HBM bandwidth)
            Bt = big.tile([D, R], bf16)
            nc.sync.dma_start(Bt[:], embB[:])
            A = [big.tile([D, 1024], bf16, name=f"A{j}") for j in range(8)]
            Mk = [big.tile([128, WINW], fp8, name=f"Mk{j}") for j in range(MC)]
            Id = big.tile([128, 128], fp8)
            nc.sync.dma_start(A[0][:], embA[:, 0:1024])
            nc.sync.dma_start(Id[:], iden[:])
            nc.sync.dma_start(Mk[0][:], masks[0])
            nc.sync.dma_start(A[1][:], embA[:, 1024:2048])
            nc.sync.dma_start(A[2][:], embA[:, 2048:3072])
            nc.sync.dma_start(A[3][:], embA[:, 3072:4096])
            nc.sync.dma_start(Mk[1][:], masks[1])
            nc.sync.dma_start(A[4][:], embA[:, 4096:5120])
            nc.sync.dma_start(A[5][:], embA[:, 5120:6144])
            nc.sync.dma_start(Mk[2][:], masks[2])
            nc.sync.dma_start(A[6][:], embA[:, 6144:7168])
            nc.sync.dma_start(A[7][:], embA[:, 7168:8192])
            for j in range(3, MC):
                nc.sync.dma_start(Mk[j][:], masks[j])

            min_t = st.tile([128, MC], f32)
            max_t = st.tile([128, MC, 3], f32)
            lse_t = st.tile([128, MC, 4], f32)
            bias_t = st.tile([128, 1], f32)
            nc.gpsimd.memset(bias_t[:], -T_LSE * C_LSE)

            Mps = ps.tile([128, 4096], f32)

            def fill(half, rnd, mc):
                # sim cols [rnd*4096+half*2048 : +2048] -> Mps[:, half*2048:+2048]
                lhsT = Bt[:, mc * 128:(mc + 1) * 128]
                for j in range(4):
                    col = rnd * 4096 + half * 2048 + j * 512
                    a = A[col // 1024]
                    off = col % 1024
                    win = col < WINW  # banks 0,1 of round 0: add -4*eq mask
                    dst = Mps[:, half * 2048 + j * 512: half * 2048 + (j + 1) * 512]
                    nc.tensor.matmul(dst, lhsT, a[:, off:off + 512],
                                     start=True, stop=not win)
                    if win:
                        nc.tensor.matmul(dst, Id[:],
                                         Mk[mc][:, col:col + 512],
                                         start=False, stop=True)

            def lse(out_ap, in_ap, acc_ap):
                nc.scalar.activation(out_ap, in_ap, Exp,
                                     bias=bias_t[:], scale=T_LSE,
                                     accum_out=acc_ap)

            # pre-ramp dummies: start the PE p-state ramp once embB lands
            # (chunk0's X1 j=3 start=True clears this bank anyway)
            for _ in range(N_DUM):
                nc.tensor.matmul(Mps[:, 3584:4096], Bt[:, 0:128], Bt[:, 0:512],
                                 start=True, stop=True, skip_group_check=True)

            for mc in range(MC):
                fill(0, 0, mc)   # W half:  cols [0:2048], window masked
                fill(1, 0, mc)   # X1 half: cols [2048:4096]

                # window: exact min (hardest pos - 4) + exact max, banks 0,1
                nc.vector.tensor_reduce(
                    min_t[:, mc:mc + 1], Mps[:, 0:WINW], axis=AX, op=amin)
                nc.vector.tensor_reduce(
                    max_t[:, mc, 0:1], Mps[:, 0:WINW], axis=AX, op=amax)
                # W rest: LSE on ScalarE, banks 2,3 (no bank sharing with DVE)
                s0 = sc.tile([128, 2048], bf16, tag="sc", name=f"s0_{mc}")
                lse(s0[:, 0:1024], Mps[:, WINW:2048], lse_t[:, mc, 0:1])

                # X1: LSE full half
                s1 = sc.tile([128, 2048], bf16, tag="sc", name=f"s1_{mc}")
                lse(s1[:], Mps[:, 2048:4096], lse_t[:, mc, 1:2])

                fill(0, 1, mc)   # X2 half: cols [4096:6144] -> Mps[0:2048]

                s2 = sc.tile([128, 2048], bf16, tag="sc", name=f"s2_{mc}")
                lse(s2[:, 0:X2_ACT], Mps[:, 0:X2_ACT], lse_t[:, mc, 2:3])
                nc.vector.tensor_reduce(
                    max_t[:, mc, 1:2], Mps[:, X2_ACT:2048], axis=AX, op=amax)

                fill(1, 1, mc)   # X3 half: cols [6144:8192] -> Mps[2048:4096]

                nc.vector.tensor_reduce(
                    max_t[:, mc, 2:3], Mps[:, 2048:2048 + X3_DVE],
                    axis=AX, op=amax)
                s3 = sc.tile([128, 2048], bf16, tag="sc", name=f"s3_{mc}")
                lse(s3[:, 0:4096 - 2048 - X3_DVE], Mps[:, 2048 + X3_DVE:4096],
                    lse_t[:, mc, 3:4])

            nc.sync.dma_start(mins[:], min_t[:])
            nc.sync.dma_start(maxs[:], max_t[:])
            nc.sync.dma_start(lses[:], lse_t[:])

    nc.compile()
    _CACHE["nc"] = nc
    return nc


def _pack_bins(labels, nbins=M, cap=R):
    """Assign whole classes to cores, each core exactly `cap` rows."""
    classes, counts = np.unique(labels, return_counts=True)
    order = np.argsort(-counts)
    bins = [[] for _ in range(nbins)]
    loads = [0] * nbins
    for idx in order:
        b = int(np.argmin(loads))
        bins[b].append(int(classes[idx]))
        loads[b] += int(counts[idx])
    size = {int(c): int(s) for c, s in zip(classes, counts)}
    for _ in range(100000):
        err = [l - cap for l in loads]
        if all(e == 0 for e in err):
            return bins
        over = max(range(nbins), key=lambda b: err[b])
        under = min(range(nbins), key=lambda b: err[b])
        cur = abs(err[over]) + abs(err[under])
        best = None
        for c1 in bins[over]:
            new = abs(err[over] - size[c1]) + abs(err[under] + size[c1])
            if new < cur and (best is None or new < best[0]):
                best = (new, c1, None)
        for c1 in bins[over]:
            for c2 in bins[under]:
                d = size[c1] - size[c2]
                if d <= 0:
                    continue
                new = abs(err[over] - d) + abs(err[under] + d)
                if new < cur and (best is None or new < best[0]):
                    best = (new, c1, c2)
        if best is None:
            return None
        _, c1, c2 = best
        bins[over].remove(c1)
        bins[under].append(c1)
        loads[over] -= size[c1]
        loads[under] += size[c1]
        if c2 is not None:
            bins[under].remove(c2)
            bins[over].append(c2)
            loads[under] -= size[c2]
            loads[over] += size[c2]
    return None


def _prep_inputs(emb, labels):
    """Class-pack rows into cores, rotate columns, build fp8 window masks."""
    import ml_dtypes

    emb = np.asarray(emb, dtype=np.float32)
    labels = np.asarray(labels)

    bins = _pack_bins(labels)
    assert bins is not None, "class bin packing failed"
    # row order: bin by bin, grouped by class inside each bin
    cls_rows = {}
    srt = np.argsort(labels, kind="stable")
    slab = labels[srt]
    bounds = np.searchsorted(slab, np.arange(int(labels.max()) + 2))
    for c in np.unique(labels):
        cls_rows[int(c)] = srt[bounds[c]:bounds[c + 1]]
    order = np.concatenate(
        [np.concatenate([cls_rows[c] for c in bins[b]]) for b in range(M)]
    )
    labs = labels[order]
    embs = emb[order]
    embT = np.ascontiguousarray(embs.T)  # [D, B]

    _, counts_all = np.unique(labels, return_counts=True)
    cnt_of = {int(c): int(s) for c, s in
              zip(*np.unique(labels, return_counts=True))}
    valid = np.array([cnt_of[int(l)] >= 2 for l in labs], dtype=bool)

    iden = np.eye(128, dtype=ml_dtypes.float8_e4m3)

    in_maps = []
    for c in range(M):
        r0 = c * R
        perm = (r0 + np.arange(B)) % B
        embA = np.ascontiguousarray(embT[:, perm]).astype(ml_dtypes.bfloat16)
        embB = np.ascontiguousarray(embT[:, r0:r0 + R]).astype(ml_dtypes.bfloat16)
        lab_rows = labs[r0:r0 + R].reshape(MC, 128)
        lab_win = labs[r0:r0 + R]
        eq = lab_rows[:, :, None] == lab_win[None, None, :]
        mk = np.where(eq, np.float32(-4.0), np.float32(0.0)).astype(
            ml_dtypes.float8_e4m3
        )
        in_maps.append({"embA": embA, "embB": embB, "masks": mk, "iden": iden})
    return in_maps, valid


def _postprocess(results, valid):
    minv = np.zeros(B, dtype=np.float32)
    maxv = np.zeros(B, dtype=np.float32)
    for c, res in enumerate(results):
        mn = res["mins"]                       # [128, MC]
        mx = res["maxs"].max(axis=2)           # [128, MC]
        ls = res["lses"].astype(np.float64).sum(axis=2)
        with np.errstate(divide="ignore"):
            lse = C_LSE + np.log(ls) / T_LSE
        hn_sim = np.maximum(mx, lse.astype(np.float32))
        for mc in range(MC):
            rows = slice(c * R + mc * 128, c * R + mc * 128 + 128)
            minv[rows] = mn[:, mc]
            maxv[rows] = hn_sim[:, mc]
    hp = 1.0 - (minv + 4.0)   # hardest positive distance
    hn = 1.0 - maxv           # hardest negative distance
    per_row = np.maximum(0.0, hp - hn + MARGIN)
    cnt = int(valid.sum())
    if cnt == 0:
        return np.float32(0.0)
    return np.float32(np.sum(per_row[valid]) / cnt)


def run_device(in_maps, trace=False):
    from concourse.bass_utils import run_bass_kernel_spmd

    nc = _build_program()
    return run_bass_kernel_spmd(nc, in_maps, list(range(M)), trace=trace)


def kernel(emb, labels):
    in_maps, valid = _prep_inputs(emb, labels)
    out = run_device(in_maps, trace=False)
    return _postprocess(out.results, valid)


if __name__ == "__main__":
    rng = np.random.default_rng(0)
    emb = rng.standard_normal((B, D)).astype(np.float32)
    emb /= np.linalg.norm(emb, axis=1, keepdims=True) + 1e-12
    labels = rng.integers(0, 512, B).astype(np.int32)
    print(kernel(emb, labels))
